# revision 57
# baseline (speedup 1.0000x reference)
"""Multi-head attention (dense transformer block) on 8 Trainium2 NeuronCores.

Reference computation (per batch element b of 8):
    qkv = x @ w_qkv.T + b_qkv                  # [1024, 2304]
    q, k, v = split heads (12 heads, d=64)
    attn = softmax(q k^T / sqrt(d))
    out  = (attn v) reshaped @ w_proj.T + b_proj
Sharding: pure data parallel — core b handles batch element b, weights are
replicated, no collectives.

Per-core kernel (all matmul operands fp16, fp32 PSUM accumulation):
  C: v    = x Wv^T + b_v            -> [1024, 12*(64+1)] (ones col per head
     makes the PV matmul emit softmax row-sums for free)
  B: qk^T = [Wq*scale; Wk] x^T      -> [1536, 1024] (features on partitions)
  D: per head pair hp: scores^T = k^T q (two heads row-tiled into the two
     halves of the PE array), exp on ScalarE straight from PSUM pairs,
     PV accumulate, normalize by approx-reciprocal(rowsum).
  E: out = score w_proj^T + b_proj  (bias via pre-broadcast rows on DVE)

Schedule notes (v2):
  - All DRAM inputs are host-prearranged partition-major so every DMA is a
    flat 2-3 level descriptor; issue is split across both HWDGE issuing
    engines (sync: x + wv, scalar: wqk + wp + biases) with the pieces the
    first matmuls need queued first, so the PE starts ~5us earlier.
  - B(0)/B(6) run first (their data lands first); all C groups become
    wave-0/1 fillers.
  - pv fillers are spread through each wave (alternating with B fillers)
    so their PSUM-evict chains never bunch up and stall the PE.
  - Wave 5 computes the nq=512 scores first so the pv(5) normalize chains
    are hidden under the first e-tiles; out staging+DMA is fp16 (host
    casts back to fp32).
"""

import os
import sys

for _p in ("/opt/trn_rl_repo", "/root/.axon_site/_ro/trn_rl_repo"):
    if os.path.isdir(_p) and _p not in sys.path:
        sys.path.insert(0, _p)

import numpy as np

import concourse.bass as bass
import concourse.mybir as mybir
import concourse.tile as tile
from concourse import bacc
from concourse.bass_utils import run_bass_kernel_spmd

DIM = 768
N_HEAD = 12
HEAD_DIM = 64
SCALE = HEAD_DIM ** (-0.5)
NB = 8          # batch == number of cores
N = 1024        # sequence length
CCH = DIM // 128  # 6 contraction chunks

F32 = mybir.dt.float32
F16 = mybir.dt.float16
AF = mybir.ActivationFunctionType

_CACHE: dict = {}


def _build():
    nc = bacc.Bacc("TRN2", target_bir_lowering=False, debug=False)

    # partition-major inputs, pre-arranged on host so every DMA below reads
    # and writes fully contiguous per-partition runs:
    #   x_h   [p, nq, c, 512]   wqk_h [p, ot, c, 128]
    #   wv_h / wp_h: blk0 [p, c, 512] then blk1 [p, c, 256], flattened
    x_d = nc.dram_tensor("x_p", [128, CCH * N], F16, kind="ExternalInput")
    wqk_d = nc.dram_tensor("wqk_p", [128, CCH * 2 * DIM], F16, kind="ExternalInput")
    wv_d = nc.dram_tensor("wv_p", [128, CCH * DIM], F16, kind="ExternalInput")
    wp_d = nc.dram_tensor("wp_p", [128, CCH * DIM], F16, kind="ExternalInput")
    # cols 0:12 = b_qk per o-tile, 12:18 = b_proj per o-tile
    bias_d = nc.dram_tensor("biases", [128, 18], F32, kind="ExternalInput")
    bv_d = nc.dram_tensor("b_v", [1, DIM], F16, kind="ExternalInput")
    out_d = nc.dram_tensor("outT", [DIM, N], F16, kind="ExternalOutput")

    with tile.TileContext(nc) as tc:
        with (
            tc.tile_pool(name="consts", bufs=1) as consts,
            tc.tile_pool(name="qk", bufs=1) as qk_pool,
            tc.tile_pool(name="score", bufs=1) as score_pool,
            tc.tile_pool(name="v", bufs=1) as v_pool,
            tc.tile_pool(name="x", bufs=1) as x_pool,
            tc.tile_pool(name="wqk", bufs=1) as wqk_pool,
            tc.tile_pool(name="wv", bufs=1) as wv_pool,
            tc.tile_pool(name="wp", bufs=1) as wp_pool,
            tc.tile_pool(name="attn", bufs=32) as attn_pool,
            tc.tile_pool(name="small", bufs=4) as small_pool,
            tc.tile_pool(name="ostage", bufs=1) as out_pool,
            tc.tile_pool(name="ps", bufs=2, space="PSUM") as ps_pool,
            tc.tile_pool(name="pair", bufs=2, space="PSUM") as pair_pool,
            tc.tile_pool(name="acc", bufs=2, space="PSUM") as acc_pool,
        ):
            x_sb = x_pool.tile([128, 2, CCH, 512], F16)      # [p, nq, c, n]
            wqk_sb = wqk_pool.tile([128, 12, CCH, 128], F16)  # [p, ot, c, o]
            wva_sb = wv_pool.tile([128, CCH, 512], F16)
            wvb_sb = wv_pool.tile([128, CCH, 256], F16)
            wp_sb = wp_pool.tile([128, CCH, CCH, 128], F16)   # [p, ot, c, o]
            bias_sb = consts.tile([128, 18], F32)
            bv_sb = consts.tile([1, DIM], F16)

            # scalar-engine HWDGE queue: the qk-projection weights the first
            # B groups need, then biases + v weights, then the rest
            wqk_v = wqk_d[:].rearrange("p (t c o) -> p t c o", t=12, c=CCH)
            wv_v = wv_d[:]
            nc.scalar.dma_start(wqk_sb[:, 0], wqk_v[:, 0])
            nc.scalar.dma_start(wqk_sb[:, 6], wqk_v[:, 6])
            nc.scalar.dma_start(bias_sb[:], bias_d[:])
            nc.scalar.dma_start(bv_sb[:], bv_d[:])
            nc.scalar.dma_start(
                wva_sb[:], wv_v[:, 0:CCH * 512].rearrange("p (c o) -> p c o", c=CCH))
            nc.scalar.dma_start(wqk_sb[:, 1:6], wqk_v[:, 1:6])
            nc.scalar.dma_start(wqk_sb[:, 7:12], wqk_v[:, 7:12])
            nc.scalar.dma_start(
                wp_sb[:], wp_d[:].rearrange("p (t c o) -> p t c o", t=CCH, c=CCH))

            # sync-engine HWDGE queue: x, the nq=0 half piped per c-chunk so
            # the first B group's accumulation starts on the first chunk
            x_v = x_d[:].rearrange("p (q c n) -> p q c n", q=2, c=CCH)
            for c in range(4):
                nc.sync.dma_start(x_sb[:, 0, c:c + 1], x_v[:, 0, c:c + 1])
            nc.sync.dma_start(x_sb[:, 0, 4:CCH], x_v[:, 0, 4:CCH])
            nc.sync.dma_start(x_sb[:, 1], x_v[:, 1])
            nc.sync.dma_start(
                wvb_sb[:],
                wv_v[:, CCH * 512:CCH * DIM].rearrange("p (c o) -> p c o", c=CCH))

            qk_sb = qk_pool.tile([128, 12, N], F16)         # [o=1536, n]
            score_sb = score_pool.tile([128, CCH, N], F16)  # [c=768, n]
            v_sb = v_pool.tile([128, 8, N_HEAD * 65], F16)  # [n, h*(64+1)]

            # ---- Phase C: v projection, natural layout + ones cols ----
            # (the ones memset runs on the idle gpsimd, and the b_v partition
            # broadcast is emitted after the upfront B groups, so neither
            # head-of-line blocks the DVE evicts the first score pairs need)
            v_ones = v_sb[:].rearrange("p n (h d) -> p n h d", d=65)[:, :, :, 64:65]
            nc.gpsimd.memset(v_ones, 1.0)
            bv32 = consts.tile([1, DIM], F32)
            bvb = consts.tile([128, DIM], F32)

            def c_gen(nt, blk):
                """The v-proj group as two half-units (3 matmuls each) so the
                wave scheduler can interleave one filler per score pair."""
                o0, ow, off = ((0, 512, 0), (512, 256, 8 * 65))[blk]
                wblk = (wva_sb, wvb_sb)[blk]
                nqi, n0 = nt // 4, (nt % 4) * 128
                ps = None

                def mm(c):
                    nc.tensor.matmul(
                        ps[:, :ow],
                        x_sb[:, nqi, c, n0:n0 + 128],
                        wblk[:, c, :],
                        start=(c == 0),
                        stop=(c == CCH - 1),
                    )

                def part1():
                    nonlocal ps
                    ps = ps_pool.tile([128, 512], F32)
                    for c in range(3):
                        mm(c)

                def part2():
                    for c in range(3, CCH):
                        mm(c)
                    nh = ow // 64
                    src = ps[:, :ow].rearrange("p (h d) -> p h d", d=64)
                    bias = bvb[:, o0:o0 + ow].rearrange("p (h d) -> p h d", d=64)
                    dst = v_sb[:, nt, off:off + nh * 65].rearrange(
                        "p (h d) -> p h d", d=65
                    )[:, :, 0:64]
                    nc.vector.tensor_add(dst, src, bias)

                return [part1, part2]

            def c_group(nt, blk):
                for part in c_gen(nt, blk):
                    part()

            # ---- Phase B helper: one [o-tile, nq] strip of the qk^T proj ----
            def b_gen(ot, nq):
                ps = None

                def mm(c):
                    nc.tensor.matmul(
                        ps[:],
                        wqk_sb[:, ot, c, :],
                        x_sb[:, nq // 512, c, :],
                        start=(c == 0),
                        stop=(c == CCH - 1),
                    )

                def part1():
                    nonlocal ps
                    ps = ps_pool.tile([128, 512], F32)
                    for c in range(3):
                        mm(c)

                def part2():
                    for c in range(3, CCH):
                        mm(c)
                    nc.vector.tensor_scalar_add(
                        qk_sb[:, ot, nq:nq + 512], ps[:], bias_sb[:, ot:ot + 1],
                    )

                return [part1, part2]

            def b_group(ot, nq):
                for part in b_gen(ot, nq):
                    part()

            # ---- Phase D helpers ----
            def score_pair(hp, nq, nk):
                """scoresT for both heads of pair hp, one nk tile: head A into
                cols 0:512 (PE rows 0-63), head B into 512:1024 (rows 64-127),
                then exp straight from the 2-bank PSUM pair into fp16 SBUF."""
                pair = pair_pool.tile([128, 1024], F32)
                for half, p0 in ((0, 0), (1, 64)):
                    nc.tensor.matmul(
                        pair[:, half * 512:(half + 1) * 512],
                        qk_sb[p0:p0 + 64, 6 + hp, nk * 128:(nk + 1) * 128],
                        qk_sb[p0:p0 + 64, hp, nq:nq + 512],
                        start=True, stop=True,
                        tile_position=(p0, 0),
                    )
                at = attn_pool.tile([128, 1024], F16)
                nc.scalar.activation(at[:], pair[:], AF.Exp)
                return at

            def pv_gen(hp, nq, half, p0, attns, defer_mul=False):
                """attn @ [v|1] for one head/nq strip + normalize by rowsum,
                as two half-units (4 accumulation matmuls each)."""
                h = 2 * hp + half
                acc = None

                def mm(nk):
                    nc.tensor.matmul(
                        acc[:],
                        v_sb[:, nk, h * 65:(h + 1) * 65],
                        attns[nk][:, half * 512:(half + 1) * 512],
                        start=(nk == 0),
                        stop=(nk == 7),
                    )

                def part1():
                    nonlocal acc
                    acc = acc_pool.tile([65, 512], F32)
                    for nk in range(4):
                        mm(nk)

                def part2():
                    for nk in range(4, 8):
                        mm(nk)
                    # custom-DVE ops mis-read PSUM APs at partition offsets
                    # > 0 — stage the rowsum row to SBUF first.
                    rs = small_pool.tile([1, 512], F32, tag="rs")
                    nc.vector.tensor_copy(rs[:], acc[64:65, :])
                    rec = small_pool.tile([1, 512], F32, tag="rec")
                    nc.vector.reciprocal_approx_fast(rec[:], rs[:])
                    bc = small_pool.tile([64, 512], F32, tag="bc")
                    nc.gpsimd.partition_broadcast(bc[:], rec[:], channels=64)

                    # the mul waits on the gpsimd bcast; on the in-order DVE
                    # it head-of-line blocks anything emitted after it, so
                    # the tail callers defer it past the next e-unit eviction
                    def mul():
                        nc.vector.tensor_mul(
                            score_sb[p0:p0 + 64, hp, nq:nq + 512],
                            acc[0:64, :], bc[:],
                        )
                    return mul if defer_mul else mul()

                return [part1, part2]

            def pv_group(hp, nq, half, p0, attns, defer_mul=False):
                parts = pv_gen(hp, nq, half, p0, attns, defer_mul)
                parts[0]()
                return parts[1]()

            # ---- Phases B + D interleaved in waves over head pairs.
            # Fillers are half-units (3-4 matmuls) interleaved one-or-two per
            # score pair so the scalar engine's exp never falls cumulatively
            # behind the pair stream (pair_pool is only double-buffered) ----
            b_group(0, 0)
            b_group(6, 0)
            nc.vector.tensor_copy(bv32[:], bv_sb[:])
            nc.gpsimd.partition_broadcast(bvb[:], bv32[:], channels=128)

            def paced(slots, units):
                """Emit the score pairs with filler units spread evenly.
                A unit may return a closure (a deferred pv normalize-mul);
                it is emitted after the NEXT unit so its gpsimd-bcast wait
                never head-of-line blocks that unit's DVE eviction."""
                strips = []
                nu, ns, popped = len(units), len(slots), 0
                held = None
                for si, s in enumerate(slots):
                    strips.append(score_pair(*s))
                    target = nu if si + 1 == ns else (si + 1) * nu // ns
                    while popped < target:
                        ret = units[popped]()
                        popped += 1
                        if held is not None:
                            held()
                        held = ret if callable(ret) else None
                if held is not None:
                    held()
                return strips

            prev_strips = None
            for hp in range(4):
                units = []
                if hp == 0:
                    # the nq=512 B strips run as the first wave-0 fillers:
                    # the early (0,0,nk<4) pairs need only B(0,0)/B(6,0), so
                    # the PE needn't drain these before scoring starts, and
                    # B(6,512) is evicted just in time for the nk>=4 pairs
                    units += b_gen(0, 512) + b_gen(6, 512)
                    for blk in (0, 1):
                        for nt in range(6):
                            units += c_gen(nt, blk)
                elif hp == 1:
                    # remaining v-proj tiles run before this wave's pv(0)
                    # fillers, so v_sb is complete when PV needs it
                    for nt in (6, 7):
                        for blk in (0, 1):
                            units += c_gen(nt, blk)
                pu, bu = [], []
                for nq in (0, 512):
                    for ot in (hp + 1, 7 + hp):
                        bu += b_gen(ot, nq)
                if prev_strips is not None:
                    php, pstrips = prev_strips
                    for nq in (0, 512):
                        for half, p0 in ((0, 0), (1, 64)):
                            pu += pv_gen(php, nq, half, p0, pstrips[nq],
                                         defer_mul=True)
                while pu or bu:
                    if pu:
                        units.append(pu.pop(0))
                    if bu:
                        units.append(bu.pop(0))
                strips = {0: [], 512: []}
                slots = [(hp, nq, nk) for nq in (0, 512) for nk in range(8)]
                for st, (_, nq, _) in zip(paced(slots, units), slots):
                    strips[nq].append(st)
                prev_strips = (hp, strips)

            # ---- Phase E: transposed out-proj, one [o-tile, nq] unit at a
            # time: wp tile stationary, score moving, bias per-partition ----
            outT_sb = out_pool.tile([128, CCH, N], F16)

            def e_unit(ot, nq):
                ps = ps_pool.tile([128, 512], F32)
                for c in range(CCH):
                    nc.tensor.matmul(
                        ps[:],
                        wp_sb[:, ot, c, :],
                        score_sb[:, c, nq:nq + 512],
                        start=(c == 0),
                        stop=(c == CCH - 1),
                    )
                nc.scalar.activation(
                    outT_sb[:, ot, nq:nq + 512], ps[:], AF.Identity,
                    bias=bias_sb[:, 12 + ot:13 + ot],
                )
                nc.sync.dma_start(
                    out_d[ot * 128:(ot + 1) * 128, nq:nq + 512],
                    outT_sb[:, ot, nq:nq + 512],
                )

            # ---- wave 4: the nq=512 half interleaves hp=5's nq=512 score
            # pairs (their B groups are this wave's fillers), so wave 5 has
            # only 8 pairs left and enough pv fillers to stay PE-bound ----
            _, p3 = prev_strips
            front = (pv_gen(3, 0, 0, 0, p3[0], defer_mul=True) + b_gen(5, 0)
                     + pv_gen(3, 0, 1, 64, p3[0], defer_mul=True)
                     + b_gen(5, 512))
            strips4 = {0: paced([(4, 0, nk) for nk in range(8)], front)}
            # b(11) half-units lead so the pulled hp=5 pairs' keys are ready
            back = (b_gen(11, 0) + b_gen(11, 512)
                    + pv_gen(3, 512, 0, 0, p3[512], defer_mul=True)
                    + pv_gen(3, 512, 1, 64, p3[512], defer_mul=True))
            slots = ([(4, 512, 0), (4, 512, 1), (4, 512, 2), (4, 512, 3),
                      (4, 512, 4), (5, 512, 0), (4, 512, 5), (5, 512, 1),
                      (4, 512, 6), (5, 512, 2), (4, 512, 7), (5, 512, 3)]
                     + [(5, 512, nk) for nk in range(4, 8)])
            got = paced(slots, back)
            strips4[512] = [st for st, s in zip(got, slots) if s[0] == 4]
            strips5 = {512: [st for st, s in zip(got, slots) if s[0] == 5]}

            # ---- wave 5: remaining hp=5 nq=0 pairs with pv(4)/pv(5,512)
            # fillers (pv(5,512) early so its chains land before e units) ----
            w5 = (pv_gen(4, 512, 0, 0, strips4[512], defer_mul=True)
                  + pv_gen(5, 512, 0, 0, strips5[512], defer_mul=True)
                  + pv_gen(4, 512, 1, 64, strips4[512], defer_mul=True)
                  + pv_gen(5, 512, 1, 64, strips5[512], defer_mul=True)
                  + pv_gen(4, 0, 0, 0, strips4[0], defer_mul=True)
                  + pv_gen(4, 0, 1, 64, strips4[0], defer_mul=True))
            strips5[0] = paced([(5, 0, nk) for nk in range(8)], w5)
            e_unit(0, 512)
            mul0 = pv_group(5, 0, 0, 0, strips5[0], defer_mul=True)
            e_unit(1, 512)
            mul1 = pv_group(5, 0, 1, 64, strips5[0], defer_mul=True)
            e_unit(2, 512)
            mul0()
            e_unit(3, 512)
            mul1()
            e_unit(4, 512)
            e_unit(5, 512)
            for ot in range(CCH):
                e_unit(ot, 0)

    nc.compile()
    return nc


def _get_nc():
    if "nc" not in _CACHE:
        _CACHE["nc"] = _build()
    return _CACHE["nc"]


def _x_h(xT):
    """[768, 1024] -> [p, nq, c, 512] flattened fp16."""
    return np.ascontiguousarray(
        xT.reshape(CCH, 128, 2, 512).transpose(1, 2, 0, 3)
    ).reshape(128, CCH * N).astype(np.float16)


def _wqk_h(w):
    """[768, 1536] -> [p, ot, c, 128] flattened fp16."""
    return np.ascontiguousarray(
        w.reshape(CCH, 128, 12, 128).transpose(1, 2, 0, 3)
    ).reshape(128, CCH * 2 * DIM).astype(np.float16)


def _wblk_h(w):
    """[768, 768] -> blk0 [p, c, 512] ++ blk1 [p, c, 256] fp16."""
    b0 = w[:, 0:512].reshape(CCH, 128, 512).transpose(1, 0, 2).reshape(128, -1)
    b1 = w[:, 512:DIM].reshape(CCH, 128, 256).transpose(1, 0, 2).reshape(128, -1)
    return np.ascontiguousarray(
        np.concatenate([b0, b1], axis=1)).astype(np.float16)


def _wp_h(w):
    """[768, 768] -> [p, ot, c, 128] flattened fp16."""
    return np.ascontiguousarray(
        w.reshape(CCH, 128, CCH, 128).transpose(1, 2, 0, 3)
    ).reshape(128, CCH * DIM).astype(np.float16)


def kernel(x, w_qkv, b_qkv, w_proj, b_proj, **run_kwargs):
    x = np.asarray(x, dtype=np.float32)
    w_qkv = np.asarray(w_qkv, dtype=np.float32)
    b_qkv = np.asarray(b_qkv, dtype=np.float32)
    w_proj = np.asarray(w_proj, dtype=np.float32)
    b_proj = np.asarray(b_proj, dtype=np.float32)

    # Host-side layout prep (no arithmetic beyond folding the 1/sqrt(d) scale
    # into the q projection).
    w_qk = w_qkv[: 2 * DIM].copy()
    b_qk = b_qkv[: 2 * DIM].copy()
    w_qk[:DIM] *= SCALE
    b_qk[:DIM] *= SCALE
    wqk_p = _wqk_h(w_qk.T)                             # [128, 6*1536]
    wv_p = _wblk_h(w_qkv[2 * DIM:].T)
    wp_p = _wp_h(w_proj.T)
    biases = np.concatenate(
        [b_qk.reshape(12, 128).T, b_proj.reshape(CCH, 128).T], axis=1)
    biases = np.ascontiguousarray(biases).astype(np.float32)   # [128, 18]
    b_v = b_qkv[2 * DIM:].reshape(1, DIM).astype(np.float16)

    nc = _get_nc()
    in_maps = []
    for b in range(NB):
        in_maps.append({
            "x_p": _x_h(x[b].T),
            "wqk_p": wqk_p,
            "biases": biases,
            "wv_p": wv_p,
            "b_v": b_v,
            "wp_p": wp_p,
        })
    res = run_bass_kernel_spmd(nc, in_maps, core_ids=list(range(NB)), **run_kwargs)
    out = np.stack(
        [res.results[b]["outT"].T for b in range(NB)], axis=0).astype(np.float32)
    if run_kwargs:
        return out, res
    return out


if __name__ == "__main__":
    rng = np.random.default_rng(0)
    x = rng.standard_normal((NB, N, DIM), dtype=np.float32)
    w_qkv = rng.standard_normal((3 * DIM, DIM), dtype=np.float32) * DIM ** -0.5
    b_qkv = rng.standard_normal((3 * DIM,), dtype=np.float32) * 0.02
    w_proj = rng.standard_normal((DIM, DIM), dtype=np.float32) * DIM ** -0.5
    b_proj = rng.standard_normal((DIM,), dtype=np.float32) * 0.02
    out = kernel(x=x, w_qkv=w_qkv, b_qkv=b_qkv, w_proj=w_proj, b_proj=b_proj)
    print("out", out.shape, out.dtype, float(np.abs(out).mean()))


# revision 58
# speedup vs baseline: 1.1982x; 1.1982x over previous
"""Multi-head attention (dense transformer block) on 8 Trainium2 NeuronCores.

Reference computation (per batch element b of 8):
    qkv = x @ w_qkv.T + b_qkv                  # [1024, 2304]
    q, k, v = split heads (12 heads, d=64)
    attn = softmax(q k^T / sqrt(d))
    out  = (attn v) reshaped @ w_proj.T + b_proj
Sharding: pure data parallel — core b handles batch element b, weights are
replicated, no collectives.

Per-core kernel (all matmul operands fp16, fp32 PSUM accumulation):
  C: v    = x Wv^T + b_v            -> [1024, 12*(64+1)] (ones col per head
     makes the PV matmul emit softmax row-sums for free)
  B: qk^T = [Wq*scale; Wk] x^T      -> [1536, 1024] (features on partitions)
  D: per head pair hp: scores^T = k^T q (two heads row-tiled into the two
     halves of the PE array), exp on ScalarE straight from PSUM pairs,
     PV accumulate, normalize by approx-reciprocal(rowsum).
  E: out = score w_proj^T + b_proj  (bias via pre-broadcast rows on DVE)

Schedule notes (v2):
  - All DRAM inputs are host-prearranged partition-major so every DMA is a
    flat 2-3 level descriptor; issue is split across both HWDGE issuing
    engines (sync: x + wv, scalar: wqk + wp + biases) with the pieces the
    first matmuls need queued first, so the PE starts ~5us earlier.
  - B(0)/B(6) run first (their data lands first); all C groups become
    wave-0/1 fillers.
  - pv fillers are spread through each wave (alternating with B fillers)
    so their PSUM-evict chains never bunch up and stall the PE.
  - Wave 5 computes the nq=512 scores first so the pv(5) normalize chains
    are hidden under the first e-tiles; out staging+DMA is fp16 (host
    casts back to fp32).
"""

import os
import sys

for _p in ("/opt/trn_rl_repo", "/root/.axon_site/_ro/trn_rl_repo"):
    if os.path.isdir(_p) and _p not in sys.path:
        sys.path.insert(0, _p)

import numpy as np

import concourse.bass as bass
import concourse.mybir as mybir
import concourse.tile as tile
from concourse import bacc
from concourse.bass_utils import run_bass_kernel_spmd

DIM = 768
N_HEAD = 12
HEAD_DIM = 64
SCALE = HEAD_DIM ** (-0.5)
NB = 8          # batch == number of cores
N = 1024        # sequence length
CCH = DIM // 128  # 6 contraction chunks

F32 = mybir.dt.float32
F16 = mybir.dt.float16
AF = mybir.ActivationFunctionType

_CACHE: dict = {}


def _build():
    nc = bacc.Bacc("TRN2", target_bir_lowering=False, debug=False)

    # partition-major inputs, pre-arranged on host so every DMA below reads
    # and writes fully contiguous per-partition runs:
    #   x_h   [p, nq, c, 512]   wqk_h [p, ot, c, 128]
    #   wv_h / wp_h: blk0 [p, c, 512] then blk1 [p, c, 256], flattened
    x_d = nc.dram_tensor("x_p", [128, CCH * N], F16, kind="ExternalInput")
    wqk_d = nc.dram_tensor("wqk_p", [128, CCH * 2 * DIM], F16, kind="ExternalInput")
    wv_d = nc.dram_tensor("wv_p", [128, CCH * DIM], F16, kind="ExternalInput")
    wp_d = nc.dram_tensor("wp_p", [128, CCH * DIM], F16, kind="ExternalInput")
    # cols 0:12 = b_qk per o-tile, 12:18 = b_proj per o-tile
    bias_d = nc.dram_tensor("biases", [128, 18], F32, kind="ExternalInput")
    bv_d = nc.dram_tensor("b_v", [1, DIM], F16, kind="ExternalInput")
    out_d = nc.dram_tensor("outT", [DIM, N], F16, kind="ExternalOutput")

    with tile.TileContext(nc) as tc:
        with (
            tc.tile_pool(name="consts", bufs=1) as consts,
            tc.tile_pool(name="qk", bufs=1) as qk_pool,
            tc.tile_pool(name="score", bufs=1) as score_pool,
            tc.tile_pool(name="v", bufs=1) as v_pool,
            tc.tile_pool(name="x", bufs=1) as x_pool,
            tc.tile_pool(name="wqk", bufs=1) as wqk_pool,
            tc.tile_pool(name="wv", bufs=1) as wv_pool,
            tc.tile_pool(name="wp", bufs=1) as wp_pool,
            tc.tile_pool(name="attn", bufs=32) as attn_pool,
            tc.tile_pool(name="small", bufs=4) as small_pool,
            tc.tile_pool(name="ostage", bufs=1) as out_pool,
            tc.tile_pool(name="ps", bufs=2, space="PSUM") as ps_pool,
            tc.tile_pool(name="pair", bufs=2, space="PSUM") as pair_pool,
            tc.tile_pool(name="acc", bufs=2, space="PSUM") as acc_pool,
        ):
            x_sb = x_pool.tile([128, 2, CCH, 512], F16)      # [p, nq, c, n]
            wqk_sb = wqk_pool.tile([128, 12, CCH, 128], F16)  # [p, ot, c, o]
            wva_sb = wv_pool.tile([128, CCH, 512], F16)
            wvb_sb = wv_pool.tile([128, CCH, 256], F16)
            wp_sb = wp_pool.tile([128, CCH, CCH, 128], F16)   # [p, ot, c, o]
            bias_sb = consts.tile([128, 18], F32)
            bv_sb = consts.tile([1, DIM], F16)

            # scalar-engine HWDGE queue: the qk-projection weights the first
            # B groups need, then biases + v weights, then the rest
            wqk_v = wqk_d[:].rearrange("p (t c o) -> p t c o", t=12, c=CCH)
            wv_v = wv_d[:]
            nc.scalar.dma_start(wqk_sb[:, 0], wqk_v[:, 0])
            nc.scalar.dma_start(wqk_sb[:, 6], wqk_v[:, 6])
            nc.scalar.dma_start(bias_sb[:], bias_d[:])
            nc.scalar.dma_start(bv_sb[:], bv_d[:])
            nc.scalar.dma_start(
                wva_sb[:], wv_v[:, 0:CCH * 512].rearrange("p (c o) -> p c o", c=CCH))
            nc.scalar.dma_start(wqk_sb[:, 1:6], wqk_v[:, 1:6])
            nc.scalar.dma_start(wqk_sb[:, 7:12], wqk_v[:, 7:12])
            nc.scalar.dma_start(
                wp_sb[:], wp_d[:].rearrange("p (t c o) -> p t c o", t=CCH, c=CCH))

            # sync-engine HWDGE queue: x, the nq=0 half piped per c-chunk so
            # the first B group's accumulation starts on the first chunk
            x_v = x_d[:].rearrange("p (q c n) -> p q c n", q=2, c=CCH)
            for c in range(3):
                nc.sync.dma_start(x_sb[:, 0, c:c + 1], x_v[:, 0, c:c + 1])
            nc.sync.dma_start(x_sb[:, 0, 3:CCH], x_v[:, 0, 3:CCH])
            nc.sync.dma_start(x_sb[:, 1], x_v[:, 1])
            nc.sync.dma_start(
                wvb_sb[:],
                wv_v[:, CCH * 512:CCH * DIM].rearrange("p (c o) -> p c o", c=CCH))

            qk_sb = qk_pool.tile([128, 12, N], F16)         # [o=1536, n]
            score_sb = score_pool.tile([128, CCH, N], F16)  # [c=768, n]
            v_sb = v_pool.tile([128, 8, N_HEAD * 65], F16)  # [n, h*(64+1)]

            # ---- Phase C: v projection, natural layout + ones cols ----
            # (the ones memset runs on the idle gpsimd, and the b_v partition
            # broadcast is emitted after the upfront B groups, so neither
            # head-of-line blocks the DVE evicts the first score pairs need)
            v_ones = v_sb[:].rearrange("p n (h d) -> p n h d", d=65)[:, :, :, 64:65]
            nc.gpsimd.memset(v_ones, 1.0)
            bv32 = consts.tile([1, DIM], F32)
            bvb = consts.tile([128, DIM], F32)

            def c_gen(nt, blk):
                """The v-proj group as two half-units (3 matmuls each) so the
                wave scheduler can interleave one filler per score pair."""
                o0, ow, off = ((0, 512, 0), (512, 256, 8 * 65))[blk]
                wblk = (wva_sb, wvb_sb)[blk]
                nqi, n0 = nt // 4, (nt % 4) * 128
                ps = None

                def mm(c):
                    nc.tensor.matmul(
                        ps[:, :ow],
                        x_sb[:, nqi, c, n0:n0 + 128],
                        wblk[:, c, :],
                        start=(c == 0),
                        stop=(c == CCH - 1),
                    )

                def part1():
                    nonlocal ps
                    ps = ps_pool.tile([128, 512], F32)
                    for c in range(3):
                        mm(c)

                def part2():
                    for c in range(3, CCH):
                        mm(c)
                    nh = ow // 64
                    src = ps[:, :ow].rearrange("p (h d) -> p h d", d=64)
                    bias = bvb[:, o0:o0 + ow].rearrange("p (h d) -> p h d", d=64)
                    dst = v_sb[:, nt, off:off + nh * 65].rearrange(
                        "p (h d) -> p h d", d=65
                    )[:, :, 0:64]
                    nc.vector.tensor_add(dst, src, bias)

                return [part1, part2]

            def c_group(nt, blk):
                for part in c_gen(nt, blk):
                    part()

            # ---- Phase B helper: one [o-tile, nq] strip of the qk^T proj ----
            def b_gen(ot, nq):
                ps = None

                def mm(c):
                    nc.tensor.matmul(
                        ps[:],
                        wqk_sb[:, ot, c, :],
                        x_sb[:, nq // 512, c, :],
                        start=(c == 0),
                        stop=(c == CCH - 1),
                    )

                def part1():
                    nonlocal ps
                    ps = ps_pool.tile([128, 512], F32)
                    for c in range(3):
                        mm(c)

                def part2():
                    for c in range(3, CCH):
                        mm(c)
                    nc.vector.tensor_scalar_add(
                        qk_sb[:, ot, nq:nq + 512], ps[:], bias_sb[:, ot:ot + 1],
                    )

                return [part1, part2]

            def b_group(ot, nq):
                for part in b_gen(ot, nq):
                    part()

            # ---- Phase D helpers ----
            def score_pair(hp, nq, nk):
                """scoresT for both heads of pair hp, one nk tile: head A into
                cols 0:512 (PE rows 0-63), head B into 512:1024 (rows 64-127),
                then exp straight from the 2-bank PSUM pair into fp16 SBUF."""
                pair = pair_pool.tile([128, 1024], F32)
                for half, p0 in ((0, 0), (1, 64)):
                    nc.tensor.matmul(
                        pair[:, half * 512:(half + 1) * 512],
                        qk_sb[p0:p0 + 64, 6 + hp, nk * 128:(nk + 1) * 128],
                        qk_sb[p0:p0 + 64, hp, nq:nq + 512],
                        start=True, stop=True,
                        tile_position=(p0, 0),
                    )
                at = attn_pool.tile([128, 1024], F16)
                nc.scalar.activation(at[:], pair[:], AF.Exp)
                return at

            def pv_gen(hp, nq, half, p0, attns, defer_mul=False):
                """attn @ [v|1] for one head/nq strip + normalize by rowsum,
                as two half-units (4 accumulation matmuls each)."""
                h = 2 * hp + half
                acc = None

                def mm(nk):
                    nc.tensor.matmul(
                        acc[:],
                        v_sb[:, nk, h * 65:(h + 1) * 65],
                        attns[nk][:, half * 512:(half + 1) * 512],
                        start=(nk == 0),
                        stop=(nk == 7),
                    )

                def part1():
                    nonlocal acc
                    acc = acc_pool.tile([65, 512], F32)
                    for nk in range(4):
                        mm(nk)

                def part2():
                    for nk in range(4, 8):
                        mm(nk)
                    # custom-DVE ops mis-read PSUM APs at partition offsets
                    # > 0 — stage the rowsum row to SBUF first.
                    rs = small_pool.tile([1, 512], F32, tag="rs")
                    nc.vector.tensor_copy(rs[:], acc[64:65, :])
                    rec = small_pool.tile([1, 512], F32, tag="rec")
                    nc.vector.reciprocal_approx_fast(rec[:], rs[:])
                    bc = small_pool.tile([64, 512], F32, tag="bc")
                    nc.gpsimd.partition_broadcast(bc[:], rec[:], channels=64)

                    # the mul waits on the gpsimd bcast; on the in-order DVE
                    # it head-of-line blocks anything emitted after it, so
                    # the tail callers defer it past the next e-unit eviction
                    def mul():
                        nc.vector.tensor_mul(
                            score_sb[p0:p0 + 64, hp, nq:nq + 512],
                            acc[0:64, :], bc[:],
                        )
                    return mul if defer_mul else mul()

                return [part1, part2]

            def pv_group(hp, nq, half, p0, attns, defer_mul=False):
                parts = pv_gen(hp, nq, half, p0, attns, defer_mul)
                parts[0]()
                return parts[1]()

            # ---- Phases B + D interleaved in waves over head pairs.
            # Fillers are half-units (3-4 matmuls) interleaved one-or-two per
            # score pair so the scalar engine's exp never falls cumulatively
            # behind the pair stream (pair_pool is only double-buffered) ----
            b_group(0, 0)
            b_group(6, 0)
            nc.vector.tensor_copy(bv32[:], bv_sb[:])
            nc.gpsimd.partition_broadcast(bvb[:], bv32[:], channels=128)

            def paced(slots, units):
                """Emit the score pairs with filler units spread evenly.
                A unit may return a closure (a deferred pv normalize-mul);
                it is emitted after the NEXT unit so its gpsimd-bcast wait
                never head-of-line blocks that unit's DVE eviction."""
                strips = []
                nu, ns, popped = len(units), len(slots), 0
                held = None
                for si, s in enumerate(slots):
                    strips.append(score_pair(*s))
                    target = nu if si + 1 == ns else (si + 1) * nu // ns
                    while popped < target:
                        ret = units[popped]()
                        popped += 1
                        if held is not None:
                            held()
                        held = ret if callable(ret) else None
                if held is not None:
                    held()
                return strips

            prev_strips = None
            for hp in range(4):
                units = []
                if hp == 0:
                    # the nq=512 B strips run as the first wave-0 fillers:
                    # the early (0,0,nk<4) pairs need only B(0,0)/B(6,0), so
                    # the PE needn't drain these before scoring starts, and
                    # B(6,512) is evicted just in time for the nk>=4 pairs
                    units += b_gen(0, 512) + b_gen(6, 512)
                    for blk in (0, 1):
                        for nt in range(6):
                            units += c_gen(nt, blk)
                elif hp == 1:
                    # remaining v-proj tiles run before this wave's pv(0)
                    # fillers, so v_sb is complete when PV needs it
                    for nt in (6, 7):
                        for blk in (0, 1):
                            units += c_gen(nt, blk)
                pu, bu = [], []
                for nq in (0, 512):
                    for ot in (hp + 1, 7 + hp):
                        bu += b_gen(ot, nq)
                if prev_strips is not None:
                    php, pstrips = prev_strips
                    for nq in (0, 512):
                        for half, p0 in ((0, 0), (1, 64)):
                            pu += pv_gen(php, nq, half, p0, pstrips[nq],
                                         defer_mul=True)
                while pu or bu:
                    if pu:
                        units.append(pu.pop(0))
                    if bu:
                        units.append(bu.pop(0))
                strips = {0: [], 512: []}
                slots = [(hp, nq, nk) for nq in (0, 512) for nk in range(8)]
                for st, (_, nq, _) in zip(paced(slots, units), slots):
                    strips[nq].append(st)
                prev_strips = (hp, strips)

            # ---- Phase E: transposed out-proj, one [o-tile, nq] unit at a
            # time: wp tile stationary, score moving, bias per-partition ----
            outT_sb = out_pool.tile([128, CCH, N], F16)

            def e_unit(ot, nq):
                ps = ps_pool.tile([128, 512], F32)
                for c in range(CCH):
                    nc.tensor.matmul(
                        ps[:],
                        wp_sb[:, ot, c, :],
                        score_sb[:, c, nq:nq + 512],
                        start=(c == 0),
                        stop=(c == CCH - 1),
                    )
                nc.scalar.activation(
                    outT_sb[:, ot, nq:nq + 512], ps[:], AF.Identity,
                    bias=bias_sb[:, 12 + ot:13 + ot],
                )
                nc.sync.dma_start(
                    out_d[ot * 128:(ot + 1) * 128, nq:nq + 512],
                    outT_sb[:, ot, nq:nq + 512],
                )

            # ---- wave 4: the nq=512 half interleaves hp=5's nq=512 score
            # pairs (their B groups are this wave's fillers), so wave 5 has
            # only 8 pairs left and enough pv fillers to stay PE-bound ----
            _, p3 = prev_strips
            front = (pv_gen(3, 0, 0, 0, p3[0], defer_mul=True) + b_gen(5, 0)
                     + pv_gen(3, 0, 1, 64, p3[0], defer_mul=True)
                     + b_gen(5, 512))
            strips4 = {0: paced([(4, 0, nk) for nk in range(8)], front)}
            # b(11) half-units lead so the pulled hp=5 pairs' keys are ready
            back = (b_gen(11, 0) + b_gen(11, 512)
                    + pv_gen(3, 512, 0, 0, p3[512], defer_mul=True)
                    + pv_gen(3, 512, 1, 64, p3[512], defer_mul=True))
            slots = ([(4, 512, 0), (4, 512, 1), (4, 512, 2), (4, 512, 3),
                      (4, 512, 4), (5, 512, 0), (4, 512, 5), (5, 512, 1),
                      (4, 512, 6), (5, 512, 2), (4, 512, 7), (5, 512, 3)]
                     + [(5, 512, nk) for nk in range(4, 8)])
            got = paced(slots, back)
            strips4[512] = [st for st, s in zip(got, slots) if s[0] == 4]
            strips5 = {512: [st for st, s in zip(got, slots) if s[0] == 5]}

            # ---- wave 5: remaining hp=5 nq=0 pairs with pv(4)/pv(5,512)
            # fillers (pv(5,512) early so its chains land before e units) ----
            w5 = (pv_gen(4, 512, 0, 0, strips4[512], defer_mul=True)
                  + pv_gen(5, 512, 0, 0, strips5[512], defer_mul=True)
                  + pv_gen(4, 512, 1, 64, strips4[512], defer_mul=True)
                  + pv_gen(5, 512, 1, 64, strips5[512], defer_mul=True)
                  + pv_gen(4, 0, 0, 0, strips4[0], defer_mul=True)
                  + pv_gen(4, 0, 1, 64, strips4[0], defer_mul=True))
            strips5[0] = paced([(5, 0, nk) for nk in range(8)], w5)
            e_unit(0, 512)
            mul0 = pv_group(5, 0, 0, 0, strips5[0], defer_mul=True)
            e_unit(1, 512)
            mul1 = pv_group(5, 0, 1, 64, strips5[0], defer_mul=True)
            e_unit(2, 512)
            mul0()
            e_unit(3, 512)
            mul1()
            e_unit(4, 512)
            e_unit(5, 512)
            for ot in range(CCH):
                e_unit(ot, 0)

    nc.compile()
    return nc


def _get_nc():
    if "nc" not in _CACHE:
        _CACHE["nc"] = _build()
    return _CACHE["nc"]


def _x_h(xT):
    """[768, 1024] -> [p, nq, c, 512] flattened fp16."""
    return np.ascontiguousarray(
        xT.reshape(CCH, 128, 2, 512).transpose(1, 2, 0, 3)
    ).reshape(128, CCH * N).astype(np.float16)


def _wqk_h(w):
    """[768, 1536] -> [p, ot, c, 128] flattened fp16."""
    return np.ascontiguousarray(
        w.reshape(CCH, 128, 12, 128).transpose(1, 2, 0, 3)
    ).reshape(128, CCH * 2 * DIM).astype(np.float16)


def _wblk_h(w):
    """[768, 768] -> blk0 [p, c, 512] ++ blk1 [p, c, 256] fp16."""
    b0 = w[:, 0:512].reshape(CCH, 128, 512).transpose(1, 0, 2).reshape(128, -1)
    b1 = w[:, 512:DIM].reshape(CCH, 128, 256).transpose(1, 0, 2).reshape(128, -1)
    return np.ascontiguousarray(
        np.concatenate([b0, b1], axis=1)).astype(np.float16)


def _wp_h(w):
    """[768, 768] -> [p, ot, c, 128] flattened fp16."""
    return np.ascontiguousarray(
        w.reshape(CCH, 128, CCH, 128).transpose(1, 2, 0, 3)
    ).reshape(128, CCH * DIM).astype(np.float16)


def kernel(x, w_qkv, b_qkv, w_proj, b_proj, **run_kwargs):
    x = np.asarray(x, dtype=np.float32)
    w_qkv = np.asarray(w_qkv, dtype=np.float32)
    b_qkv = np.asarray(b_qkv, dtype=np.float32)
    w_proj = np.asarray(w_proj, dtype=np.float32)
    b_proj = np.asarray(b_proj, dtype=np.float32)

    # Host-side layout prep (no arithmetic beyond folding the 1/sqrt(d) scale
    # into the q projection).
    w_qk = w_qkv[: 2 * DIM].copy()
    b_qk = b_qkv[: 2 * DIM].copy()
    w_qk[:DIM] *= SCALE
    b_qk[:DIM] *= SCALE
    wqk_p = _wqk_h(w_qk.T)                             # [128, 6*1536]
    wv_p = _wblk_h(w_qkv[2 * DIM:].T)
    wp_p = _wp_h(w_proj.T)
    biases = np.concatenate(
        [b_qk.reshape(12, 128).T, b_proj.reshape(CCH, 128).T], axis=1)
    biases = np.ascontiguousarray(biases).astype(np.float32)   # [128, 18]
    b_v = b_qkv[2 * DIM:].reshape(1, DIM).astype(np.float16)

    nc = _get_nc()
    in_maps = []
    for b in range(NB):
        in_maps.append({
            "x_p": _x_h(x[b].T),
            "wqk_p": wqk_p,
            "biases": biases,
            "wv_p": wv_p,
            "b_v": b_v,
            "wp_p": wp_p,
        })
    res = run_bass_kernel_spmd(nc, in_maps, core_ids=list(range(NB)), **run_kwargs)
    out = np.stack(
        [res.results[b]["outT"].T for b in range(NB)], axis=0).astype(np.float32)
    if run_kwargs:
        return out, res
    return out


if __name__ == "__main__":
    rng = np.random.default_rng(0)
    x = rng.standard_normal((NB, N, DIM), dtype=np.float32)
    w_qkv = rng.standard_normal((3 * DIM, DIM), dtype=np.float32) * DIM ** -0.5
    b_qkv = rng.standard_normal((3 * DIM,), dtype=np.float32) * 0.02
    w_proj = rng.standard_normal((DIM, DIM), dtype=np.float32) * DIM ** -0.5
    b_proj = rng.standard_normal((DIM,), dtype=np.float32) * 0.02
    out = kernel(x=x, w_qkv=w_qkv, b_qkv=b_qkv, w_proj=w_proj, b_proj=b_proj)
    print("out", out.shape, out.dtype, float(np.abs(out).mean()))
